# revision 29
# baseline (speedup 1.0000x reference)
"""AdaptiveSoftmax (moe_routing) Trainium2 kernel — 8-core data-parallel.

Reference computes:
  head = x @ head_w.T + head_b                      [8192, 2002]
  out0 = ((x @ t0w1.T) @ t0w2.T) * mask0[:, None]   [8192, 8000]
  out1 = ((x @ t1w1.T) @ t1w2.T) * mask1[:, None]   [8192, 20000]
with mask_i selecting rows whose target falls in cluster i.

Strategy: the routing (masks) depends only on `target`, which is known
host-side, and the chains are linear — so gather the cluster rows on the
host, compute dense compact matmuls on-device (data-parallel over rows on
8 NeuronCores), and scatter back into zero-filled full outputs.

Stage-1 matmuls (h = x @ w1.T) run with the small w1 stationary and x.T
moving, producing h.T feature-major. Stage-2 / head matmuls run with
x.T/h.T tiles stationary and the big weights moving (amortizes
LDWEIGHTS, keeps every matmul at the max 512 moving columns) and produce
row-major outputs directly. Compute in bf16 (f32 PSUM accumulate),
outputs stored bf16, upcast + bias-add host-side.
"""

import math
import os

import numpy as np
import ml_dtypes

import concourse.bass as bass
import concourse.mybir as mybir
import concourse.tile as tile
from concourse import bacc
from concourse.bass_utils import run_bass_kernel_spmd


def _ensure_ntff_hook():
    """bass_utils' trace path does `from antenv.axon_hooks import ...`;
    some images ship antenv without that submodule. Register a shim wired
    to the boot helper so tracing works (or degrades to None) instead of
    raising ImportError."""
    try:
        import antenv.axon_hooks  # noqa: F401
        return
    except ImportError:
        pass
    import sys
    import types
    try:
        import antenv
    except ImportError:
        return
    try:
        from trn_agent_boot.trn_boot import _ntff_profile_via_ctypes
        hook = _ntff_profile_via_ctypes("/opt/axon/libaxon_pjrt.so")
    except Exception:
        hook = None
    mod = types.ModuleType("antenv.axon_hooks")
    mod._hook = hook
    mod.get_axon_ntff_profile_hook = lambda: mod._hook

    def _set(h):
        mod._hook = h
    mod.set_axon_ntff_profile_hook = _set
    sys.modules["antenv.axon_hooks"] = mod
    antenv.axon_hooks = mod


_ensure_ntff_hook()

N_CORES = 8
D = 1024
HEAD_OUT = 2002
HEAD_PAD = 2048
OUT0 = 8000
OUT0_PAD = 8064
OUT1 = 20000
OUT1_PAD = 20096
D1 = 256
CUT = (2000, 10000, 30000)

BF16 = mybir.dt.bfloat16
F32 = mybir.dt.float32
NPBF16 = ml_dtypes.bfloat16

_NEFF_CACHE: dict = {}


def _ceil_to(v: int, m: int) -> int:
    return ((v + m - 1) // m) * m


def _chunks(total: int, size: int):
    return [(i * size, min(size, total - i * size))
            for i in range(math.ceil(total / size))]


def _swizzle_stat(w: np.ndarray, m_pad: int) -> np.ndarray:
    """[M, K] weight -> [Mt, 128, Kt*128] bf16: per m-tile, the K-on-partition
    transposed tile, k-tiles along free dim (for stationary use)."""
    M, K = w.shape
    wp = np.zeros((m_pad, K), np.float32)
    wp[:M] = w
    Mt, Kt = m_pad // 128, K // 128
    a = wp.reshape(Mt, 128, Kt, 128)            # (mt, m, kt, p)
    b = a.transpose(0, 3, 2, 1)                 # (mt, p, kt, m)
    return np.ascontiguousarray(b.reshape(Mt, 128, Kt * 128).astype(NPBF16))


def _swizzle_mov(w: np.ndarray, m_pad: int) -> np.ndarray:
    """[M, K] weight -> [Kt, 128, m_pad] bf16 = w.T split into k-tiles
    (for moving use: K on partitions, all M on free dim)."""
    M, K = w.shape
    wp = np.zeros((m_pad, K), np.float32)
    wp[:M] = w
    t = wp.T.reshape(K // 128, 128, m_pad)
    return np.ascontiguousarray(t.astype(NPBF16))


def _shard_xT(xr: np.ndarray, rows_per_core: int) -> list[np.ndarray]:
    """[R, D] rows (R == 8*rows_per_core, zero-padded) -> per-core
    [Kt, 128, rows_per_core] bf16 transposed shards."""
    Kt = xr.shape[1] // 128
    out = []
    xb = xr.astype(NPBF16)
    for c in range(N_CORES):
        sh = xb[c * rows_per_core:(c + 1) * rows_per_core]
        t = sh.T.reshape(Kt, 128, rows_per_core)
        out.append(np.ascontiguousarray(t))
    return out


def _build(r0: int, r1: int):
    """Build + compile the per-core program."""
    rh = 8192 // N_CORES
    nc = bacc.Bacc("TRN2", target_bir_lowering=False, debug=False,
                   num_devices=N_CORES)

    xT = nc.declare_dram_parameter("xT", [8, 128, rh], BF16, isOutput=False)
    x0T = nc.declare_dram_parameter("x0T", [8, 128, r0], BF16, isOutput=False)
    x1T = nc.declare_dram_parameter("x1T", [8, 128, r1], BF16, isOutput=False)
    hw = nc.declare_dram_parameter("hw", [8, 128, HEAD_PAD], BF16, isOutput=False)
    w10 = nc.declare_dram_parameter("w10", [8, 128, 1024], BF16, isOutput=False)
    w20 = nc.declare_dram_parameter("w20", [8, 128, OUT0_PAD], BF16, isOutput=False)
    w11 = nc.declare_dram_parameter("w11", [2, 128, 1024], BF16, isOutput=False)
    w21 = nc.declare_dram_parameter("w21", [2, 128, OUT1_PAD], BF16, isOutput=False)

    head_o = nc.declare_dram_parameter("head_o", [rh, HEAD_OUT], BF16, isOutput=True)
    out0_o = nc.declare_dram_parameter("out0_o", [r0, OUT0_PAD], BF16, isOutput=True)
    out1_o = nc.declare_dram_parameter("out1_o", [r1, OUT1_PAD], BF16, isOutput=True)

    nb_h = _chunks(rh, 128)      # head row blocks (8 full)
    nb_0 = _chunks(r0, 128)      # cluster-0 row blocks
    nb_1 = _chunks(r1, 128)      # cluster-1 row blocks

    cp_flip = [0]

    def psum_copy(nc, dst, src):
        """Alternate PSUM->SBUF copies between DVE and ACT."""
        cp_flip[0] ^= 1
        if cp_flip[0]:
            nc.vector.tensor_copy(dst, src)
        else:
            nc.scalar.activation(dst, src, mybir.ActivationFunctionType.Copy)

    with tile.TileContext(nc) as tc:
        with (
            tc.tile_pool(name="xres", bufs=1) as xres,
            tc.tile_pool(name="w10p", bufs=3) as w10p,
            tc.tile_pool(name="w20p", bufs=8) as w20p,
            tc.tile_pool(name="stg", bufs=3) as stg,
            tc.tile_pool(name="ps", bufs=1, space="PSUM") as psp,
        ):
            # ---- tiles ----
            x0t = [xres.tile([128, r0], BF16, name=f"x0t{k}") for k in range(8)]
            xt = [xres.tile([128, rh], BF16, name=f"xt{k}") for k in range(8)]
            hwr = [xres.tile([128, HEAD_PAD], BF16, name=f"hwr{k}") for k in range(8)]
            x1t = [xres.tile([128, r1], BF16, name=f"x1t{k}") for k in range(8)]
            w11r = [xres.tile([128, 1024], BF16, name=f"w11r{j}") for j in range(2)]
            w21r = [xres.tile([128, OUT1_PAD], BF16, name=f"w21r{k}") for k in range(2)]
            h0t = [xres.tile([128, r0], BF16, name=f"h0t{j}") for j in range(8)]
            h1t = [xres.tile([128, r1], BF16, name=f"h1t{j}") for j in range(2)]

            # Bulk loads for later phases are split into pieces and issued
            # between stream DMAs of earlier phases, so the just-in-time
            # streams never sit behind megabytes of prefetch in the queue.
            side_a = []      # needed by head phase: xt, hwr
            for k in range(8):
                side_a.append((xt[k][:], xT[k]))
                side_a.append((hwr[k][:], hw[k]))
            side_b = []      # needed by h1/out1: w21 pieces, x1t, w11
            for k in range(2):
                for (off, w) in _chunks(OUT1_PAD, 2560):
                    side_b.append((w21r[k][:, off:off + w], w21[k][:, off:off + w]))
            for k in range(8):
                side_b.append((x1t[k][:], x1T[k]))
            for j in range(2):
                side_b.append((w11r[j][:], w11[j]))

            def issue_side(q, n):
                for _ in range(min(n, len(q))):
                    dst, src_ap = q.pop(0)
                    nc.sync.dma_start(out=dst, in_=src_ap)

            # 8 psum bank tiles, shared by tag across phases
            def ps_tile(i):
                return psp.tile([128, 512], F32, name=f"psb{i}", tag=f"psb{i}")

            # ---- HAM warmup: keep the PE busy during the initial DMA
            # window so real matmuls start at the full 2.4 GHz clock.
            # Reads uninitialized SBUF; results are never consumed.
            warm = xres.tile([128, 512], BF16, name="warmsrc")
            nc.gpsimd.memset(warm[:], 0.0)
            wps = ps_tile(6)
            for _ in range(22):
                nc.tensor.matmul(wps[:], warm[:, :128], warm[:],
                                 start=True, stop=True)

            # ---- h0 = (x0 @ t0w1.T).T : w1 stationary, x0.T moving ----
            mc0 = max(1, 8 // max(1, len(nb_0)))
            slabs0 = _chunks(OUT0_PAD, 512 * mc0)
            sw0 = slabs0[0][1]
            pre_chunks = []
            for jt in range(8):
                wt = w10p.tile([128, 1024], BF16, name="w10t", tag="w10t")
                nc.sync.dma_start(out=wt[:], in_=w10[jt])
                if jt == 0:
                    for k in range(8):
                        nc.sync.dma_start(out=x0t[k][:], in_=x0T[k])
                else:
                    # prefetch out0 slab-0 weight chunk k=jt-1 between w10 loads
                    pk = jt - 1
                    wch = w20p.tile([128, 512 * mc0], BF16, name="w20c", tag="w20c")
                    nc.sync.dma_start(out=wch[:, :sw0], in_=w20[pk][:, 0:sw0])
                    pre_chunks.append(wch)
                for ci, (o, w) in enumerate(_chunks(r0, 512)):
                    ps = ps_tile(6 + (jt + ci) % 2)
                    for k in range(8):
                        nc.tensor.matmul(ps[:, :w], wt[:, k * 128:(k + 1) * 128],
                                         x0t[k][:, o:o + w],
                                         start=(k == 0), stop=(k == 7))
                    psum_copy(nc, h0t[jt][:, o:o + w], ps[:, :w])

            # ---- out0: h0.T tiles stationary, t0w2.T moving (streamed) ----
            side_a_per = (len(side_a) + len(slabs0) - 1) // len(slabs0)
            for si, (soff, sw) in enumerate(slabs0):
                scs = _chunks(sw, 512)
                pss = {}
                for bi in range(len(nb_0)):
                    for i in range(len(scs)):
                        pss[(bi, i)] = ps_tile((2 * si + bi * mc0 + i) % 8)
                for k in range(8):
                    if si == 0 and k < len(pre_chunks):
                        wch = pre_chunks[k]
                    else:
                        wch = w20p.tile([128, 512 * mc0], BF16, name="w20c", tag="w20c")
                        nc.sync.dma_start(out=wch[:, :sw], in_=w20[k][:, soff:soff + sw])
                    if k in (3, 7):
                        issue_side(side_a, side_a_per // 2 + 1)
                    for bi, (boff, bw) in enumerate(nb_0):
                        for i, (co, cw) in enumerate(scs):
                            nc.tensor.matmul(pss[(bi, i)][:bw, :cw],
                                             h0t[k][:, boff:boff + bw],
                                             wch[:, co:co + cw],
                                             start=(k == 0), stop=(k == 7))
                for bi, (boff, bw) in enumerate(nb_0):
                    st = stg.tile([128, 512 * mc0], BF16, name="o0stg", tag="o0stg")
                    for i, (co, cw) in enumerate(scs):
                        psum_copy(nc, st[:bw, co:co + cw], pss[(bi, i)][:bw, :cw])
                    nc.sync.dma_start(out=out0_o[boff:boff + bw, soff:soff + sw],
                                      in_=st[:bw, :sw])
            issue_side(side_a, len(side_a))

            # ---- head rows: x.T tiles stationary, head_w.T moving ----
            side_b_per = (len(side_b) + len(nb_h) - 1) // len(nb_h)
            for nbi, (boff, bw) in enumerate(nb_h):
                mcs = _chunks(HEAD_OUT, 512)                 # 4 chunks (last 466)
                base = (nbi % 2) * 4
                pss = [ps_tile(base + i) for i in range(len(mcs))]
                for k in range(8):
                    for i, (mo, mw) in enumerate(mcs):
                        nc.tensor.matmul(pss[i][:bw, :mw],
                                         xt[k][:, boff:boff + bw],
                                         hwr[k][:, mo:mo + mw],
                                         start=(k == 0), stop=(k == 7))
                st = stg.tile([128, HEAD_OUT], BF16, name="hstg", tag="hstg")
                for i, (mo, mw) in enumerate(mcs):
                    psum_copy(nc, st[:bw, mo:mo + mw], pss[i][:bw, :mw])
                nc.sync.dma_start(out=head_o[boff:boff + bw, :], in_=st[:bw, :])
                issue_side(side_b, side_b_per)
            issue_side(side_b, len(side_b))

            # ---- h1 = (x1 @ t1w1.T).T ----
            for jt in range(2):
                for ci, (o, w) in enumerate(_chunks(r1, 512)):
                    ps = ps_tile(6 + (jt + ci) % 2)
                    for k in range(8):
                        nc.tensor.matmul(ps[:, :w], w11r[jt][:, k * 128:(k + 1) * 128],
                                         x1t[k][:, o:o + w],
                                         start=(k == 0), stop=(k == 7))
                    psum_copy(nc, h1t[jt][:, o:o + w], ps[:, :w])

            # ---- out1: h1.T tiles stationary, t1w2.T moving (resident) ----
            slabs1 = _chunks(OUT1_PAD, 2048)
            for nbi, (boff, bw) in enumerate(nb_1):
                for si, (soff, sw) in enumerate(slabs1):
                    scs = _chunks(sw, 512)
                    base = ((nbi * len(slabs1) + si) % 2) * 4
                    pss = [ps_tile(base + i) for i in range(len(scs))]
                    for k in range(2):
                        for i, (co, cw) in enumerate(scs):
                            nc.tensor.matmul(pss[i][:bw, :cw],
                                             h1t[k][:, boff:boff + bw],
                                             w21r[k][:, soff + co:soff + co + cw],
                                             start=(k == 0), stop=(k == 1))
                    st = stg.tile([128, 2048], BF16, name="o1stg", tag="o1stg")
                    for i, (co, cw) in enumerate(scs):
                        psum_copy(nc, st[:bw, co:co + cw], pss[i][:bw, :cw])
                    nc.sync.dma_start(out=out1_o[boff:boff + bw, soff:soff + sw],
                                      in_=st[:bw, :sw])

    nc.compile()
    return nc


def kernel(x, target, head_w, head_b, tail0_w1, tail0_w2, tail1_w1, tail1_w2):
    x = np.asarray(x, np.float32)
    target = np.asarray(target)
    n = x.shape[0]
    rh = n // N_CORES

    # --- routing (host) ---
    idx0 = np.where((target >= CUT[0]) & (target < CUT[1]))[0]
    idx1 = np.where((target >= CUT[1]) & (target < CUT[2]))[0]
    n0, n1 = len(idx0), len(idx1)
    r0 = max(64, _ceil_to(n0, N_CORES * 64) // N_CORES)
    r1 = max(64, _ceil_to(n1, N_CORES * 64) // N_CORES)

    xg0 = np.zeros((N_CORES * r0, D), np.float32)
    xg0[:n0] = x[idx0]
    xg1 = np.zeros((N_CORES * r1, D), np.float32)
    xg1[:n1] = x[idx1]

    # --- pack operands (host) ---
    xT_sh = _shard_xT(x, rh)
    x0T_sh = _shard_xT(xg0, r0)
    x1T_sh = _shard_xT(xg1, r1)
    hw_m = _swizzle_mov(np.asarray(head_w, np.float32), HEAD_PAD)
    w10_s = _swizzle_stat(np.asarray(tail0_w1, np.float32), 1024)
    w20_m = _swizzle_mov(np.asarray(tail0_w2, np.float32), OUT0_PAD)
    w11_s = _swizzle_stat(np.asarray(tail1_w1, np.float32), D1)
    w21_m = _swizzle_mov(np.asarray(tail1_w2, np.float32), OUT1_PAD)

    key = (r0, r1)
    if key not in _NEFF_CACHE:
        _NEFF_CACHE[key] = _build(r0, r1)
    nc = _NEFF_CACHE[key]

    in_maps = []
    for c in range(N_CORES):
        in_maps.append({
            "xT": xT_sh[c], "x0T": x0T_sh[c], "x1T": x1T_sh[c],
            "hw": hw_m, "w10": w10_s, "w20": w20_m,
            "w11": w11_s, "w21": w21_m,
        })

    def _finite(r):
        for c in range(N_CORES):
            for name in ("head_o", "out0_o", "out1_o"):
                if not np.all(np.isfinite(r.results[c][name].astype(np.float32))):
                    return False
        return True

    last_err = None
    res = None
    for attempt in range(4):
        try:
            res = run_bass_kernel_spmd(
                nc, in_maps, core_ids=list(range(N_CORES)),
                trace=bool(os.environ.get("KERNEL_TRACE")),
            )
            if _finite(res):
                break
            last_err = RuntimeError("non-finite device output (wedged run)")
        except Exception as e:  # transient device wedge -> retry
            last_err = e
        res = None
        import time
        time.sleep(15)
    if res is None:
        raise last_err
    kernel.last_results = res

    # --- unpack (host) ---
    head = np.concatenate([res.results[c]["head_o"] for c in range(N_CORES)])
    head = head.astype(np.float32)
    head += np.asarray(head_b, np.float32)[None, :]

    out0 = np.zeros((n, OUT0), np.float32)
    rows0 = np.concatenate([res.results[c]["out0_o"] for c in range(N_CORES)])
    out0[idx0] = rows0[:n0, :OUT0].astype(np.float32)

    out1 = np.zeros((n, OUT1), np.float32)
    rows1 = np.concatenate([res.results[c]["out1_o"] for c in range(N_CORES)])
    out1[idx1] = rows1[:n1, :OUT1].astype(np.float32)

    return head, out0, out1


# revision 30
# speedup vs baseline: 1.0196x; 1.0196x over previous
"""AdaptiveSoftmax (moe_routing) Trainium2 kernel — 8-core data-parallel.

Reference computes:
  head = x @ head_w.T + head_b                      [8192, 2002]
  out0 = ((x @ t0w1.T) @ t0w2.T) * mask0[:, None]   [8192, 8000]
  out1 = ((x @ t1w1.T) @ t1w2.T) * mask1[:, None]   [8192, 20000]
with mask_i selecting rows whose target falls in cluster i.

Strategy: the routing (masks) depends only on `target`, which is known
host-side, and the chains are linear — so gather the cluster rows on the
host, compute dense compact matmuls on-device (data-parallel over rows on
8 NeuronCores), and scatter back into zero-filled full outputs.

Stage-1 matmuls (h = x @ w1.T) run with the small w1 stationary and x.T
moving, producing h.T feature-major. Stage-2 / head matmuls run with
x.T/h.T tiles stationary and the big weights moving (amortizes
LDWEIGHTS, keeps every matmul at the max 512 moving columns) and produce
row-major outputs directly. Compute in bf16 (f32 PSUM accumulate),
outputs stored bf16, upcast + bias-add host-side.
"""

import math
import os

import numpy as np
import ml_dtypes

import concourse.bass as bass
import concourse.mybir as mybir
import concourse.tile as tile
from concourse import bacc
from concourse.bass_utils import run_bass_kernel_spmd


def _ensure_ntff_hook():
    """bass_utils' trace path does `from antenv.axon_hooks import ...`;
    some images ship antenv without that submodule. Register a shim wired
    to the boot helper so tracing works (or degrades to None) instead of
    raising ImportError."""
    try:
        import antenv.axon_hooks  # noqa: F401
        return
    except ImportError:
        pass
    import sys
    import types
    try:
        import antenv
    except ImportError:
        return
    try:
        from trn_agent_boot.trn_boot import _ntff_profile_via_ctypes
        hook = _ntff_profile_via_ctypes("/opt/axon/libaxon_pjrt.so")
    except Exception:
        hook = None
    mod = types.ModuleType("antenv.axon_hooks")
    mod._hook = hook
    mod.get_axon_ntff_profile_hook = lambda: mod._hook

    def _set(h):
        mod._hook = h
    mod.set_axon_ntff_profile_hook = _set
    sys.modules["antenv.axon_hooks"] = mod
    antenv.axon_hooks = mod


_ensure_ntff_hook()

N_CORES = 8
D = 1024
HEAD_OUT = 2002
HEAD_PAD = 2048
OUT0 = 8000
OUT0_PAD = 8064
OUT1 = 20000
OUT1_PAD = 20096
D1 = 256
CUT = (2000, 10000, 30000)

BF16 = mybir.dt.bfloat16
F32 = mybir.dt.float32
NPBF16 = ml_dtypes.bfloat16

_NEFF_CACHE: dict = {}


def _ceil_to(v: int, m: int) -> int:
    return ((v + m - 1) // m) * m


def _chunks(total: int, size: int):
    return [(i * size, min(size, total - i * size))
            for i in range(math.ceil(total / size))]


def _swizzle_stat(w: np.ndarray, m_pad: int) -> np.ndarray:
    """[M, K] weight -> [Mt, 128, Kt*128] bf16: per m-tile, the K-on-partition
    transposed tile, k-tiles along free dim (for stationary use)."""
    M, K = w.shape
    wp = np.zeros((m_pad, K), np.float32)
    wp[:M] = w
    Mt, Kt = m_pad // 128, K // 128
    a = wp.reshape(Mt, 128, Kt, 128)            # (mt, m, kt, p)
    b = a.transpose(0, 3, 2, 1)                 # (mt, p, kt, m)
    return np.ascontiguousarray(b.reshape(Mt, 128, Kt * 128).astype(NPBF16))


def _swizzle_mov(w: np.ndarray, m_pad: int) -> np.ndarray:
    """[M, K] weight -> [Kt, 128, m_pad] bf16 = w.T split into k-tiles
    (for moving use: K on partitions, all M on free dim)."""
    M, K = w.shape
    wp = np.zeros((m_pad, K), np.float32)
    wp[:M] = w
    t = wp.T.reshape(K // 128, 128, m_pad)
    return np.ascontiguousarray(t.astype(NPBF16))


def _shard_xT(xr: np.ndarray, rows_per_core: int) -> list[np.ndarray]:
    """[R, D] rows (R == 8*rows_per_core, zero-padded) -> per-core
    [Kt, 128, rows_per_core] bf16 transposed shards."""
    Kt = xr.shape[1] // 128
    out = []
    xb = xr.astype(NPBF16)
    for c in range(N_CORES):
        sh = xb[c * rows_per_core:(c + 1) * rows_per_core]
        t = sh.T.reshape(Kt, 128, rows_per_core)
        out.append(np.ascontiguousarray(t))
    return out


def _build(r0: int, r1: int):
    """Build + compile the per-core program."""
    rh = 8192 // N_CORES
    nc = bacc.Bacc("TRN2", target_bir_lowering=False, debug=False,
                   num_devices=N_CORES)

    xT = nc.declare_dram_parameter("xT", [8, 128, rh], BF16, isOutput=False)
    x0T = nc.declare_dram_parameter("x0T", [8, 128, r0], BF16, isOutput=False)
    x1T = nc.declare_dram_parameter("x1T", [8, 128, r1], BF16, isOutput=False)
    hw = nc.declare_dram_parameter("hw", [8, 128, HEAD_PAD], BF16, isOutput=False)
    w10 = nc.declare_dram_parameter("w10", [8, 128, 1024], BF16, isOutput=False)
    w20 = nc.declare_dram_parameter("w20", [8, 128, OUT0_PAD], BF16, isOutput=False)
    w11 = nc.declare_dram_parameter("w11", [2, 128, 1024], BF16, isOutput=False)
    w21 = nc.declare_dram_parameter("w21", [2, 128, OUT1_PAD], BF16, isOutput=False)

    head_o = nc.declare_dram_parameter("head_o", [rh, HEAD_OUT], BF16, isOutput=True)
    out0_o = nc.declare_dram_parameter("out0_o", [r0, OUT0_PAD], BF16, isOutput=True)
    out1_o = nc.declare_dram_parameter("out1_o", [r1, OUT1_PAD], BF16, isOutput=True)

    nb_h = _chunks(rh, 128)      # head row blocks (8 full)
    nb_0 = _chunks(r0, 128)      # cluster-0 row blocks
    nb_1 = _chunks(r1, 128)      # cluster-1 row blocks

    cp_flip = [0]

    def psum_copy(nc, dst, src):
        """Alternate PSUM->SBUF copies between DVE and ACT."""
        cp_flip[0] ^= 1
        if cp_flip[0]:
            nc.vector.tensor_copy(dst, src)
        else:
            nc.scalar.activation(dst, src, mybir.ActivationFunctionType.Copy)

    with tile.TileContext(nc) as tc:
        with (
            tc.tile_pool(name="xres", bufs=1) as xres,
            tc.tile_pool(name="w10p", bufs=3) as w10p,
            tc.tile_pool(name="w20p", bufs=8) as w20p,
            tc.tile_pool(name="stg", bufs=3) as stg,
            tc.tile_pool(name="ps", bufs=1, space="PSUM") as psp,
        ):
            # ---- tiles ----
            x0t = [xres.tile([128, r0], BF16, name=f"x0t{k}") for k in range(8)]
            xt = [xres.tile([128, rh], BF16, name=f"xt{k}") for k in range(8)]
            hwr = [xres.tile([128, HEAD_PAD], BF16, name=f"hwr{k}") for k in range(8)]
            x1t = [xres.tile([128, r1], BF16, name=f"x1t{k}") for k in range(8)]
            w11r = [xres.tile([128, 1024], BF16, name=f"w11r{j}") for j in range(2)]
            w21r = [xres.tile([128, OUT1_PAD], BF16, name=f"w21r{k}") for k in range(2)]
            h0t = [xres.tile([128, r0], BF16, name=f"h0t{j}") for j in range(8)]
            h1t = [xres.tile([128, r1], BF16, name=f"h1t{j}") for j in range(2)]

            # Bulk loads for later phases are split into pieces and issued
            # between stream DMAs of earlier phases, so the just-in-time
            # streams never sit behind megabytes of prefetch in the queue.
            side_a = []      # needed by head phase: xt, hwr
            for k in range(8):
                side_a.append((xt[k][:], xT[k]))
                side_a.append((hwr[k][:], hw[k]))
            side_b = []      # needed by h1/out1: w21 pieces, x1t, w11
            for k in range(2):
                for (off, w) in _chunks(OUT1_PAD, 2560):
                    side_b.append((w21r[k][:, off:off + w], w21[k][:, off:off + w]))
            for k in range(8):
                side_b.append((x1t[k][:], x1T[k]))
            for j in range(2):
                side_b.append((w11r[j][:], w11[j]))

            def issue_side(q, n):
                for _ in range(min(n, len(q))):
                    dst, src_ap = q.pop(0)
                    nc.sync.dma_start(out=dst, in_=src_ap)

            # 8 psum bank tiles, shared by tag across phases
            def ps_tile(i):
                return psp.tile([128, 512], F32, name=f"psb{i}", tag=f"psb{i}")

            # ---- h0 = (x0 @ t0w1.T).T : w1 stationary, x0.T moving ----
            mc0 = max(1, 8 // max(1, len(nb_0)))
            slabs0 = _chunks(OUT0_PAD, 512 * mc0)
            sw0 = slabs0[0][1]
            pre_chunks = []
            for jt in range(8):
                wt = w10p.tile([128, 1024], BF16, name="w10t", tag="w10t")
                nc.sync.dma_start(out=wt[:], in_=w10[jt])
                if jt == 0:
                    for k in range(8):
                        nc.sync.dma_start(out=x0t[k][:], in_=x0T[k])
                else:
                    # prefetch out0 slab-0 weight chunk k=jt-1 between w10 loads
                    pk = jt - 1
                    wch = w20p.tile([128, 512 * mc0], BF16, name="w20c", tag="w20c")
                    nc.sync.dma_start(out=wch[:, :sw0], in_=w20[pk][:, 0:sw0])
                    pre_chunks.append(wch)
                for ci, (o, w) in enumerate(_chunks(r0, 512)):
                    ps = ps_tile(6 + (jt + ci) % 2)
                    for k in range(8):
                        nc.tensor.matmul(ps[:, :w], wt[:, k * 128:(k + 1) * 128],
                                         x0t[k][:, o:o + w],
                                         start=(k == 0), stop=(k == 7))
                    psum_copy(nc, h0t[jt][:, o:o + w], ps[:, :w])

            # ---- out0: h0.T tiles stationary, t0w2.T moving (streamed) ----
            side_a_per = (len(side_a) + len(slabs0) - 1) // len(slabs0)
            for si, (soff, sw) in enumerate(slabs0):
                scs = _chunks(sw, 512)
                pss = {}
                for bi in range(len(nb_0)):
                    for i in range(len(scs)):
                        pss[(bi, i)] = ps_tile((2 * si + bi * mc0 + i) % 8)
                for k in range(8):
                    if si == 0 and k < len(pre_chunks):
                        wch = pre_chunks[k]
                    else:
                        wch = w20p.tile([128, 512 * mc0], BF16, name="w20c", tag="w20c")
                        nc.sync.dma_start(out=wch[:, :sw], in_=w20[k][:, soff:soff + sw])
                    if k in (3, 7):
                        issue_side(side_a, side_a_per // 2 + 1)
                    for bi, (boff, bw) in enumerate(nb_0):
                        for i, (co, cw) in enumerate(scs):
                            nc.tensor.matmul(pss[(bi, i)][:bw, :cw],
                                             h0t[k][:, boff:boff + bw],
                                             wch[:, co:co + cw],
                                             start=(k == 0), stop=(k == 7))
                for bi, (boff, bw) in enumerate(nb_0):
                    st = stg.tile([128, 512 * mc0], BF16, name="o0stg", tag="o0stg")
                    for i, (co, cw) in enumerate(scs):
                        psum_copy(nc, st[:bw, co:co + cw], pss[(bi, i)][:bw, :cw])
                    nc.sync.dma_start(out=out0_o[boff:boff + bw, soff:soff + sw],
                                      in_=st[:bw, :sw])
            issue_side(side_a, len(side_a))

            # ---- head rows: x.T tiles stationary, head_w.T moving ----
            side_b_per = (len(side_b) + len(nb_h) - 1) // len(nb_h)
            for nbi, (boff, bw) in enumerate(nb_h):
                mcs = _chunks(HEAD_OUT, 512)                 # 4 chunks (last 466)
                base = (nbi % 2) * 4
                pss = [ps_tile(base + i) for i in range(len(mcs))]
                for k in range(8):
                    for i, (mo, mw) in enumerate(mcs):
                        nc.tensor.matmul(pss[i][:bw, :mw],
                                         xt[k][:, boff:boff + bw],
                                         hwr[k][:, mo:mo + mw],
                                         start=(k == 0), stop=(k == 7))
                st = stg.tile([128, HEAD_OUT], BF16, name="hstg", tag="hstg")
                for i, (mo, mw) in enumerate(mcs):
                    psum_copy(nc, st[:bw, mo:mo + mw], pss[i][:bw, :mw])
                nc.sync.dma_start(out=head_o[boff:boff + bw, :], in_=st[:bw, :])
                issue_side(side_b, side_b_per)
            issue_side(side_b, len(side_b))

            # ---- h1 = (x1 @ t1w1.T).T ----
            for jt in range(2):
                for ci, (o, w) in enumerate(_chunks(r1, 512)):
                    ps = ps_tile(6 + (jt + ci) % 2)
                    for k in range(8):
                        nc.tensor.matmul(ps[:, :w], w11r[jt][:, k * 128:(k + 1) * 128],
                                         x1t[k][:, o:o + w],
                                         start=(k == 0), stop=(k == 7))
                    psum_copy(nc, h1t[jt][:, o:o + w], ps[:, :w])

            # ---- out1: h1.T tiles stationary, t1w2.T moving (resident) ----
            slabs1 = _chunks(OUT1_PAD, 2048)
            for nbi, (boff, bw) in enumerate(nb_1):
                for si, (soff, sw) in enumerate(slabs1):
                    scs = _chunks(sw, 512)
                    base = ((nbi * len(slabs1) + si) % 2) * 4
                    pss = [ps_tile(base + i) for i in range(len(scs))]
                    for k in range(2):
                        for i, (co, cw) in enumerate(scs):
                            nc.tensor.matmul(pss[i][:bw, :cw],
                                             h1t[k][:, boff:boff + bw],
                                             w21r[k][:, soff + co:soff + co + cw],
                                             start=(k == 0), stop=(k == 1))
                    st = stg.tile([128, 2048], BF16, name="o1stg", tag="o1stg")
                    for i, (co, cw) in enumerate(scs):
                        psum_copy(nc, st[:bw, co:co + cw], pss[i][:bw, :cw])
                    nc.sync.dma_start(out=out1_o[boff:boff + bw, soff:soff + sw],
                                      in_=st[:bw, :sw])

    nc.compile()
    return nc


def kernel(x, target, head_w, head_b, tail0_w1, tail0_w2, tail1_w1, tail1_w2):
    x = np.asarray(x, np.float32)
    target = np.asarray(target)
    n = x.shape[0]
    rh = n // N_CORES

    # --- routing (host) ---
    idx0 = np.where((target >= CUT[0]) & (target < CUT[1]))[0]
    idx1 = np.where((target >= CUT[1]) & (target < CUT[2]))[0]
    n0, n1 = len(idx0), len(idx1)
    r0 = max(64, _ceil_to(n0, N_CORES * 64) // N_CORES)
    r1 = max(64, _ceil_to(n1, N_CORES * 64) // N_CORES)

    xg0 = np.zeros((N_CORES * r0, D), np.float32)
    xg0[:n0] = x[idx0]
    xg1 = np.zeros((N_CORES * r1, D), np.float32)
    xg1[:n1] = x[idx1]

    # --- pack operands (host) ---
    xT_sh = _shard_xT(x, rh)
    x0T_sh = _shard_xT(xg0, r0)
    x1T_sh = _shard_xT(xg1, r1)
    hw_m = _swizzle_mov(np.asarray(head_w, np.float32), HEAD_PAD)
    w10_s = _swizzle_stat(np.asarray(tail0_w1, np.float32), 1024)
    w20_m = _swizzle_mov(np.asarray(tail0_w2, np.float32), OUT0_PAD)
    w11_s = _swizzle_stat(np.asarray(tail1_w1, np.float32), D1)
    w21_m = _swizzle_mov(np.asarray(tail1_w2, np.float32), OUT1_PAD)

    key = (r0, r1)
    if key not in _NEFF_CACHE:
        _NEFF_CACHE[key] = _build(r0, r1)
    nc = _NEFF_CACHE[key]

    in_maps = []
    for c in range(N_CORES):
        in_maps.append({
            "xT": xT_sh[c], "x0T": x0T_sh[c], "x1T": x1T_sh[c],
            "hw": hw_m, "w10": w10_s, "w20": w20_m,
            "w11": w11_s, "w21": w21_m,
        })

    def _finite(r):
        for c in range(N_CORES):
            for name in ("head_o", "out0_o", "out1_o"):
                if not np.all(np.isfinite(r.results[c][name].astype(np.float32))):
                    return False
        return True

    last_err = None
    res = None
    for attempt in range(4):
        try:
            res = run_bass_kernel_spmd(
                nc, in_maps, core_ids=list(range(N_CORES)),
                trace=bool(os.environ.get("KERNEL_TRACE")),
            )
            if _finite(res):
                break
            last_err = RuntimeError("non-finite device output (wedged run)")
        except Exception as e:  # transient device wedge -> retry
            last_err = e
        res = None
        import time
        time.sleep(15)
    if res is None:
        raise last_err
    kernel.last_results = res

    # --- unpack (host) ---
    head = np.concatenate([res.results[c]["head_o"] for c in range(N_CORES)])
    head = head.astype(np.float32)
    head += np.asarray(head_b, np.float32)[None, :]

    out0 = np.zeros((n, OUT0), np.float32)
    rows0 = np.concatenate([res.results[c]["out0_o"] for c in range(N_CORES)])
    out0[idx0] = rows0[:n0, :OUT0].astype(np.float32)

    out1 = np.zeros((n, OUT1), np.float32)
    rows1 = np.concatenate([res.results[c]["out1_o"] for c in range(N_CORES)])
    out1[idx1] = rows1[:n1, :OUT1].astype(np.float32)

    return head, out0, out1


# revision 32
# speedup vs baseline: 1.0556x; 1.0354x over previous
"""AdaptiveSoftmax (moe_routing) Trainium2 kernel — 8-core data-parallel.

Reference computes:
  head = x @ head_w.T + head_b                      [8192, 2002]
  out0 = ((x @ t0w1.T) @ t0w2.T) * mask0[:, None]   [8192, 8000]
  out1 = ((x @ t1w1.T) @ t1w2.T) * mask1[:, None]   [8192, 20000]
with mask_i selecting rows whose target falls in cluster i.

Strategy: the routing (masks) depends only on `target`, which is known
host-side, and the chains are linear — so gather the cluster rows on the
host, compute dense compact matmuls on-device (data-parallel over rows on
8 NeuronCores), and scatter back into zero-filled full outputs.

Stage-1 matmuls (h = x @ w1.T) run with the small w1 stationary and x.T
moving, producing h.T feature-major. Stage-2 / head matmuls run with
x.T/h.T tiles stationary and the big weights moving (amortizes
LDWEIGHTS, keeps every matmul at the max 512 moving columns) and produce
row-major outputs directly. Compute in bf16 (f32 PSUM accumulate),
outputs stored bf16, upcast + bias-add host-side.
"""

import math
import os

import numpy as np
import ml_dtypes

import concourse.bass as bass
import concourse.mybir as mybir
import concourse.tile as tile
from concourse import bacc
from concourse.bass_utils import run_bass_kernel_spmd


def _ensure_ntff_hook():
    """bass_utils' trace path does `from antenv.axon_hooks import ...`;
    some images ship antenv without that submodule. Register a shim wired
    to the boot helper so tracing works (or degrades to None) instead of
    raising ImportError."""
    try:
        import antenv.axon_hooks  # noqa: F401
        return
    except ImportError:
        pass
    import sys
    import types
    try:
        import antenv
    except ImportError:
        return
    try:
        from trn_agent_boot.trn_boot import _ntff_profile_via_ctypes
        hook = _ntff_profile_via_ctypes("/opt/axon/libaxon_pjrt.so")
    except Exception:
        hook = None
    mod = types.ModuleType("antenv.axon_hooks")
    mod._hook = hook
    mod.get_axon_ntff_profile_hook = lambda: mod._hook

    def _set(h):
        mod._hook = h
    mod.set_axon_ntff_profile_hook = _set
    sys.modules["antenv.axon_hooks"] = mod
    antenv.axon_hooks = mod


_ensure_ntff_hook()

N_CORES = 8
D = 1024
HEAD_OUT = 2002
HEAD_PAD = 2048
OUT0 = 8000
OUT0_PAD = 8064
OUT1 = 20000
OUT1_PAD = 20096
D1 = 256
CUT = (2000, 10000, 30000)

BF16 = mybir.dt.bfloat16
F32 = mybir.dt.float32
NPBF16 = ml_dtypes.bfloat16

_NEFF_CACHE: dict = {}


def _ceil_to(v: int, m: int) -> int:
    return ((v + m - 1) // m) * m


def _chunks(total: int, size: int):
    return [(i * size, min(size, total - i * size))
            for i in range(math.ceil(total / size))]


def _swizzle_stat(w: np.ndarray, m_pad: int) -> np.ndarray:
    """[M, K] weight -> [Mt, 128, Kt*128] bf16: per m-tile, the K-on-partition
    transposed tile, k-tiles along free dim (for stationary use)."""
    M, K = w.shape
    wp = np.zeros((m_pad, K), np.float32)
    wp[:M] = w
    Mt, Kt = m_pad // 128, K // 128
    a = wp.reshape(Mt, 128, Kt, 128)            # (mt, m, kt, p)
    b = a.transpose(0, 3, 2, 1)                 # (mt, p, kt, m)
    return np.ascontiguousarray(b.reshape(Mt, 128, Kt * 128).astype(NPBF16))


def _swizzle_mov(w: np.ndarray, m_pad: int) -> np.ndarray:
    """[M, K] weight -> [Kt, 128, m_pad] bf16 = w.T split into k-tiles
    (for moving use: K on partitions, all M on free dim)."""
    M, K = w.shape
    wp = np.zeros((m_pad, K), np.float32)
    wp[:M] = w
    t = wp.T.reshape(K // 128, 128, m_pad)
    return np.ascontiguousarray(t.astype(NPBF16))


def _shard_xT(xr: np.ndarray, rows_per_core: int) -> list[np.ndarray]:
    """[R, D] rows (R == 8*rows_per_core, zero-padded) -> per-core
    [Kt, 128, rows_per_core] bf16 transposed shards."""
    Kt = xr.shape[1] // 128
    out = []
    xb = xr.astype(NPBF16)
    for c in range(N_CORES):
        sh = xb[c * rows_per_core:(c + 1) * rows_per_core]
        t = sh.T.reshape(Kt, 128, rows_per_core)
        out.append(np.ascontiguousarray(t))
    return out


def _build(r0: int, r1: int):
    """Build + compile the per-core program."""
    rh = 8192 // N_CORES
    nc = bacc.Bacc("TRN2", target_bir_lowering=False, debug=False,
                   num_devices=N_CORES)

    xT = nc.declare_dram_parameter("xT", [8, 128, rh], BF16, isOutput=False)
    x0T = nc.declare_dram_parameter("x0T", [8, 128, r0], BF16, isOutput=False)
    x1T = nc.declare_dram_parameter("x1T", [8, 128, r1], BF16, isOutput=False)
    hw = nc.declare_dram_parameter("hw", [8, 128, HEAD_PAD], BF16, isOutput=False)
    w10 = nc.declare_dram_parameter("w10", [8, 128, 1024], BF16, isOutput=False)
    w20 = nc.declare_dram_parameter("w20", [8, 128, OUT0_PAD], BF16, isOutput=False)
    w11 = nc.declare_dram_parameter("w11", [2, 128, 1024], BF16, isOutput=False)
    w21 = nc.declare_dram_parameter("w21", [2, 128, OUT1_PAD], BF16, isOutput=False)

    head_o = nc.declare_dram_parameter("head_o", [rh, HEAD_OUT], BF16, isOutput=True)
    out0_o = nc.declare_dram_parameter("out0_o", [r0, OUT0_PAD], BF16, isOutput=True)
    out1_o = nc.declare_dram_parameter("out1_o", [r1, OUT1_PAD], BF16, isOutput=True)

    nb_h = _chunks(rh, 128)      # head row blocks (8 full)
    nb_0 = _chunks(r0, 128)      # cluster-0 row blocks
    nb_1 = _chunks(r1, 128)      # cluster-1 row blocks

    cp_flip = [0]

    def psum_copy(nc, dst, src):
        """Alternate PSUM->SBUF copies between DVE and ACT."""
        cp_flip[0] ^= 1
        if cp_flip[0]:
            nc.vector.tensor_copy(dst, src)
        else:
            nc.scalar.activation(dst, src, mybir.ActivationFunctionType.Copy)

    with tile.TileContext(nc) as tc:
        with (
            tc.tile_pool(name="xres", bufs=1) as xres,
            tc.tile_pool(name="w10p", bufs=3) as w10p,
            tc.tile_pool(name="w20p", bufs=8) as w20p,
            tc.tile_pool(name="stg", bufs=3) as stg,
            tc.tile_pool(name="ps", bufs=1, space="PSUM") as psp,
        ):
            # ---- tiles ----
            x0t = [xres.tile([128, r0], BF16, name=f"x0t{k}") for k in range(8)]
            xt = [xres.tile([128, rh], BF16, name=f"xt{k}") for k in range(8)]
            hwr = [xres.tile([128, HEAD_PAD], BF16, name=f"hwr{k}") for k in range(8)]
            x1t = [xres.tile([128, r1], BF16, name=f"x1t{k}") for k in range(8)]
            w11r = [xres.tile([128, 1024], BF16, name=f"w11r{j}") for j in range(2)]
            w21r = [xres.tile([128, OUT1_PAD], BF16, name=f"w21r{k}") for k in range(2)]
            h0t = [xres.tile([128, r0], BF16, name=f"h0t{j}") for j in range(8)]
            h1t = [xres.tile([128, r1], BF16, name=f"h1t{j}") for j in range(2)]

            # Bulk loads for later phases are split into pieces and issued
            # between stream DMAs of earlier phases, so the just-in-time
            # streams never sit behind megabytes of prefetch in the queue.
            side_a = []      # needed by head phase: xt, hwr
            for k in range(8):
                side_a.append((xt[k][:], xT[k]))
                side_a.append((hwr[k][:], hw[k]))
            side_b = []      # needed by h1/out1: w21 pieces, x1t, w11
            for k in range(2):
                for (off, w) in _chunks(OUT1_PAD, 2560):
                    side_b.append((w21r[k][:, off:off + w], w21[k][:, off:off + w]))
            for k in range(8):
                side_b.append((x1t[k][:], x1T[k]))
            for j in range(2):
                side_b.append((w11r[j][:], w11[j]))

            def issue_side(q, n):
                for _ in range(min(n, len(q))):
                    dst, src_ap = q.pop(0)
                    nc.sync.dma_start(out=dst, in_=src_ap)

            # 8 psum bank tiles, shared by tag across phases
            def ps_tile(i):
                return psp.tile([128, 512], F32, name=f"psb{i}", tag=f"psb{i}")

            # ---- h0 = (x0 @ t0w1.T).T : w1 stationary, x0.T moving ----
            mc0 = max(1, 8 // max(1, len(nb_0)))
            slabs0 = _chunks(OUT0_PAD, 512 * mc0)
            sw0 = slabs0[0][1]
            pre_chunks = []
            for jt in range(8):
                wt = w10p.tile([128, 1024], BF16, name="w10t", tag="w10t")
                nc.sync.dma_start(out=wt[:], in_=w10[jt])
                if jt == 0:
                    for k in range(8):
                        nc.sync.dma_start(out=x0t[k][:], in_=x0T[k])
                else:
                    # prefetch out0 slab-0 weight chunk k=jt-1 between w10 loads
                    pk = jt - 1
                    wch = w20p.tile([128, 512 * mc0], BF16, name="w20c", tag="w20c")
                    nc.sync.dma_start(out=wch[:, :sw0], in_=w20[pk][:, 0:sw0])
                    pre_chunks.append(wch)
                for ci, (o, w) in enumerate(_chunks(r0, 512)):
                    ps = ps_tile(6 + (jt + ci) % 2)
                    for k in range(8):
                        nc.tensor.matmul(ps[:, :w], wt[:, k * 128:(k + 1) * 128],
                                         x0t[k][:, o:o + w],
                                         start=(k == 0), stop=(k == 7))
                    psum_copy(nc, h0t[jt][:, o:o + w], ps[:, :w])

            # ---- out0: h0.T tiles stationary, t0w2.T moving (streamed) ----
            side_a_per = (len(side_a) + len(slabs0) - 1) // len(slabs0)
            for si, (soff, sw) in enumerate(slabs0):
                scs = _chunks(sw, 512)
                pss = {}
                for bi in range(len(nb_0)):
                    for i in range(len(scs)):
                        pss[(bi, i)] = ps_tile((2 * si + bi * mc0 + i) % 8)
                for k in range(8):
                    if si == 0 and k < len(pre_chunks):
                        wch = pre_chunks[k]
                    else:
                        wch = w20p.tile([128, 512 * mc0], BF16, name="w20c", tag="w20c")
                        nc.sync.dma_start(out=wch[:, :sw], in_=w20[k][:, soff:soff + sw])
                    if k == 3 and si < 6:
                        issue_side(side_a, 2)
                    for bi, (boff, bw) in enumerate(nb_0):
                        for i, (co, cw) in enumerate(scs):
                            nc.tensor.matmul(pss[(bi, i)][:bw, :cw],
                                             h0t[k][:, boff:boff + bw],
                                             wch[:, co:co + cw],
                                             start=(k == 0), stop=(k == 7))
                for bi, (boff, bw) in enumerate(nb_0):
                    st = stg.tile([128, 512 * mc0], BF16, name="o0stg", tag="o0stg")
                    for i, (co, cw) in enumerate(scs):
                        psum_copy(nc, st[:bw, co:co + cw], pss[(bi, i)][:bw, :cw])
                    nc.sync.dma_start(out=out0_o[boff:boff + bw, soff:soff + sw],
                                      in_=st[:bw, :sw])
            issue_side(side_a, len(side_a))

            # ---- head rows: x.T tiles stationary, head_w.T moving ----
            side_b_per = (len(side_b) + len(nb_h) - 1) // len(nb_h)
            for nbi, (boff, bw) in enumerate(nb_h):
                mcs = _chunks(HEAD_OUT, 512)                 # 4 chunks (last 466)
                base = (nbi % 2) * 4
                pss = [ps_tile(base + i) for i in range(len(mcs))]
                for k in range(8):
                    for i, (mo, mw) in enumerate(mcs):
                        nc.tensor.matmul(pss[i][:bw, :mw],
                                         xt[k][:, boff:boff + bw],
                                         hwr[k][:, mo:mo + mw],
                                         start=(k == 0), stop=(k == 7))
                st = stg.tile([128, HEAD_OUT], BF16, name="hstg", tag="hstg")
                for i, (mo, mw) in enumerate(mcs):
                    psum_copy(nc, st[:bw, mo:mo + mw], pss[i][:bw, :mw])
                nc.sync.dma_start(out=head_o[boff:boff + bw, :], in_=st[:bw, :])
                issue_side(side_b, side_b_per)
            issue_side(side_b, len(side_b))

            # ---- h1 = (x1 @ t1w1.T).T ----
            for jt in range(2):
                for ci, (o, w) in enumerate(_chunks(r1, 512)):
                    ps = ps_tile(6 + (jt + ci) % 2)
                    for k in range(8):
                        nc.tensor.matmul(ps[:, :w], w11r[jt][:, k * 128:(k + 1) * 128],
                                         x1t[k][:, o:o + w],
                                         start=(k == 0), stop=(k == 7))
                    psum_copy(nc, h1t[jt][:, o:o + w], ps[:, :w])

            # ---- out1: h1.T tiles stationary, t1w2.T moving (resident) ----
            slabs1 = _chunks(OUT1_PAD, 2048)
            for nbi, (boff, bw) in enumerate(nb_1):
                for si, (soff, sw) in enumerate(slabs1):
                    scs = _chunks(sw, 512)
                    base = ((nbi * len(slabs1) + si) % 2) * 4
                    pss = [ps_tile(base + i) for i in range(len(scs))]
                    for k in range(2):
                        for i, (co, cw) in enumerate(scs):
                            nc.tensor.matmul(pss[i][:bw, :cw],
                                             h1t[k][:, boff:boff + bw],
                                             w21r[k][:, soff + co:soff + co + cw],
                                             start=(k == 0), stop=(k == 1))
                    st = stg.tile([128, 2048], BF16, name="o1stg", tag="o1stg")
                    for i, (co, cw) in enumerate(scs):
                        psum_copy(nc, st[:bw, co:co + cw], pss[i][:bw, :cw])
                    nc.sync.dma_start(out=out1_o[boff:boff + bw, soff:soff + sw],
                                      in_=st[:bw, :sw])

    nc.compile()
    return nc


def kernel(x, target, head_w, head_b, tail0_w1, tail0_w2, tail1_w1, tail1_w2):
    x = np.asarray(x, np.float32)
    target = np.asarray(target)
    n = x.shape[0]
    rh = n // N_CORES

    # --- routing (host) ---
    idx0 = np.where((target >= CUT[0]) & (target < CUT[1]))[0]
    idx1 = np.where((target >= CUT[1]) & (target < CUT[2]))[0]
    n0, n1 = len(idx0), len(idx1)
    r0 = max(64, _ceil_to(n0, N_CORES * 64) // N_CORES)
    r1 = max(64, _ceil_to(n1, N_CORES * 64) // N_CORES)

    xg0 = np.zeros((N_CORES * r0, D), np.float32)
    xg0[:n0] = x[idx0]
    xg1 = np.zeros((N_CORES * r1, D), np.float32)
    xg1[:n1] = x[idx1]

    # --- pack operands (host) ---
    xT_sh = _shard_xT(x, rh)
    x0T_sh = _shard_xT(xg0, r0)
    x1T_sh = _shard_xT(xg1, r1)
    hw_m = _swizzle_mov(np.asarray(head_w, np.float32), HEAD_PAD)
    w10_s = _swizzle_stat(np.asarray(tail0_w1, np.float32), 1024)
    w20_m = _swizzle_mov(np.asarray(tail0_w2, np.float32), OUT0_PAD)
    w11_s = _swizzle_stat(np.asarray(tail1_w1, np.float32), D1)
    w21_m = _swizzle_mov(np.asarray(tail1_w2, np.float32), OUT1_PAD)

    key = (r0, r1)
    if key not in _NEFF_CACHE:
        _NEFF_CACHE[key] = _build(r0, r1)
    nc = _NEFF_CACHE[key]

    in_maps = []
    for c in range(N_CORES):
        in_maps.append({
            "xT": xT_sh[c], "x0T": x0T_sh[c], "x1T": x1T_sh[c],
            "hw": hw_m, "w10": w10_s, "w20": w20_m,
            "w11": w11_s, "w21": w21_m,
        })

    def _finite(r):
        for c in range(N_CORES):
            for name in ("head_o", "out0_o", "out1_o"):
                if not np.all(np.isfinite(r.results[c][name].astype(np.float32))):
                    return False
        return True

    last_err = None
    res = None
    for attempt in range(4):
        try:
            res = run_bass_kernel_spmd(
                nc, in_maps, core_ids=list(range(N_CORES)),
                trace=bool(os.environ.get("KERNEL_TRACE")),
            )
            if _finite(res):
                break
            last_err = RuntimeError("non-finite device output (wedged run)")
        except Exception as e:  # transient device wedge -> retry
            last_err = e
        res = None
        import time
        time.sleep(15)
    if res is None:
        raise last_err
    kernel.last_results = res

    # --- unpack (host) ---
    head = np.concatenate([res.results[c]["head_o"] for c in range(N_CORES)])
    head = head.astype(np.float32)
    head += np.asarray(head_b, np.float32)[None, :]

    out0 = np.zeros((n, OUT0), np.float32)
    rows0 = np.concatenate([res.results[c]["out0_o"] for c in range(N_CORES)])
    out0[idx0] = rows0[:n0, :OUT0].astype(np.float32)

    out1 = np.zeros((n, OUT1), np.float32)
    rows1 = np.concatenate([res.results[c]["out1_o"] for c in range(N_CORES)])
    out1[idx1] = rows1[:n1, :OUT1].astype(np.float32)

    return head, out0, out1
